# revision 12
# baseline (speedup 1.0000x reference)
"""Trainium2 Bass kernel for nn_Contrast_loss (B=8192, D=256, 100 classes).

Math: with mask = -same + 0.5*(1-same) + I and same_ii = 1,
    loss = 0.5*||s||^2 - 1.5*sum_c ||g_c||^2 + sum_i ||f_i||^2
where s = sum_i f_i and g_c = sum_{i: label_i = c} f_i.

Every term decomposes over feature columns, so feat is sharded column-wise
across the 8 cores (32 columns each); the host sums the per-core partials.
No cross-core collective is needed.

Per core, everything runs through one fp8 DoubleRow matmul stream:
  - the host re-encodes label as one-hot fp8 (exact in fp8) with an extra
    all-ones column (computes s in the same matmul), and feat as an fp8
    hi/lo pair (hi = e4m3(f), lo = e4m3(f - hi); ~8-bit mantissa total).
  - the loss is invariant under row permutation, so the host pairs up
    same-label rows and places each pair in one (partition, chunk-pair)
    slot. Both k-subtiles of a DoubleRow matmul then share ONE one-hot
    matrix (stride-0 weight AP), halving the one-hot DMA. Odd class counts
    leave <=100 singletons, which exactly fill one final mixed pair that
    gets two distinct matrices (sum of per-class counts is even, so
    pairs + mixed slots == B/256 exactly).
  - the PE accumulates G = [onehot|1]^T @ [hi|lo] over 32 DoubleRow matmuls.
  - the diag term sum ||f_i||^2 = sum hi^2 + sum lo^2 (the 2*hi*lo cross
    term is ~2e-5 of the total, dropped) comes from Scalar
    square-accumulate passes over fhl, overlapped with the matmul stream.
  - tail: q_c = ||g_c||^2 on DVE reading PSUM, weighted and summed with the
    diag partials, GpSimd partition-reduce, one [1,1] DMA out; the host
    sums the 8 per-core scalars.
"""

import numpy as np
import ml_dtypes

import concourse.bacc as bacc
import concourse.bass as bass
import concourse.mybir as mybir
import concourse.tile as tile
from concourse import bass_utils

B = 8192
D = 256
N_CORES = 8
DPC = D // N_CORES          # 32 feature columns per core
P = 128                     # partitions
CHUNKS = B // P             # 64 row chunks of 128
PAIRS = CHUNKS // 2         # 32 chunk pairs
NMAT = PAIRS + 1            # 31 shared matrices + 2 for the mixed pair
N_GROUPS = 4
PPG = PAIRS // N_GROUPS     # 8 pairs per group
NCLS = 100                  # label values 0..99
NR = NCLS + 12              # one-hot cols + ones col + pad to mult-of-16 (dual-fp8 LDW)
LAMDA = 0.5

FP32 = mybir.dt.float32
BF16 = mybir.dt.bfloat16
FP8 = mybir.dt.float8e4
E4M3 = ml_dtypes.float8_e4m3

_CACHED_NC = None


def _build_nc():
    nc = bacc.Bacc("TRN2", target_bir_lowering=False, debug=False,
                   num_devices=N_CORES)

    oh_d = nc.dram_tensor("oh", [P * NMAT, NR], FP8, kind="ExternalInput")
    fhl_d = nc.dram_tensor("fhl", [B, 2 * DPC], FP8, kind="ExternalInput")
    w_d = nc.dram_tensor("wv", [P, 1], FP32, kind="ExternalInput")
    out_d = nc.dram_tensor("out", [1, 1], FP32, kind="ExternalOutput")

    with tile.TileContext(nc) as tc:
        with (
            tc.tile_pool(name="big", bufs=1) as big,
            tc.tile_pool(name="small", bufs=1) as small,
            tc.tile_pool(name="psum", bufs=1, space="PSUM") as psum,
        ):
            # Row r = p*CHUNKS + k lives at (partition p, chunk k).
            oh_t = big.tile([P, NMAT, NR], FP8)
            fhl_t = big.tile([P, CHUNKS, 2 * DPC], FP8)
            sq_t = big.tile([P, CHUNKS, 2 * DPC], BF16)
            dacc = small.tile([P, N_GROUPS], FP32)
            w_t = small.tile([P, 1], FP32)
            qq = small.tile([P, 1], FP32)

            psum_g = psum.tile([NR, 2 * DPC], FP32)

            nc.scalar.dma_start(w_t[:], w_d.rearrange("p c -> p c"))
            nc.vector.memset(qq[:], 0.0)

            oh_src = oh_d.rearrange("(p t) c -> p t c", p=P)
            fhl_src = fhl_d.rearrange("(p k) d -> p k d", p=P)
            for g in range(N_GROUPS):
                tsl = slice(g * PPG, (g + 1) * PPG + (1 if g == N_GROUPS - 1
                                                      else 0))
                ksl = slice(g * 2 * PPG, (g + 1) * 2 * PPG)
                nc.sync.dma_start(oh_t[:, tsl, :], oh_src[:, tsl, :])
                nc.gpsimd.dma_start(fhl_t[:, ksl, :], fhl_src[:, ksl, :])
                # diag partials on Scalar (overlapped with the PE stream)
                nc.scalar.activation(sq_t[:, ksl, :], fhl_t[:, ksl, :],
                                     mybir.ActivationFunctionType.Square,
                                     accum_out=dacc[:, g:g + 1])
                for t in range(g * PPG, (g + 1) * PPG):
                    if t < PAIRS - 1:
                        lhsT = oh_t[:, t, :].unsqueeze(1).broadcast_to(
                            [P, 2, NR])
                    else:
                        lhsT = oh_t[:, PAIRS - 1:PAIRS + 1, :]
                    nc.tensor.matmul(psum_g[:], lhsT,
                                     fhl_t[:, 2 * t:2 * t + 2, :],
                                     start=(t == 0), stop=(t == PAIRS - 1),
                                     perf_mode=mybir.MatmulPerfMode.DoubleRow)

            # G rows: 0..99 = [g_hi | g_lo] per class, 100 = [s_hi | s_lo]
            gh = small.tile([NR, DPC], FP32)
            nc.vector.tensor_copy(gh[:], psum_g[:, 0:DPC])
            gt = small.tile([NR, DPC], FP32)
            nc.vector.tensor_add(gt[:], gh[:], psum_g[:, DPC:2 * DPC])
            qsc = small.tile([NR, DPC], FP32)
            nc.vector.tensor_mul(qsc[:], gt[:], gt[:])
            nc.vector.tensor_reduce(qq[0:NR, 0:1], qsc[:],
                                    mybir.AxisListType.X, mybir.AluOpType.add)
            dsum = small.tile([P, 1], FP32)
            nc.vector.tensor_reduce(dsum[:], dacc[:], mybir.AxisListType.X,
                                    mybir.AluOpType.add)
            comb = small.tile([P, 1], FP32)
            nc.vector.tensor_mul(comb[:], qq[:], w_t[:])
            nc.vector.tensor_add(comb[:], comb[:], dsum[:])
            res_t = small.tile([1, 1], FP32)
            nc.gpsimd.tensor_reduce(res_t[:], comb[:], mybir.AxisListType.C,
                                    mybir.AluOpType.add)
            nc.sync.dma_start(out_d[:], res_t[:])

    nc.compile()
    return nc


def _get_nc():
    global _CACHED_NC
    if _CACHED_NC is None:
        _CACHED_NC = _build_nc()
    return _CACHED_NC


def _pair_permutation(lab):
    """Assign rows to (partition, chunk) slots so that chunks 2t, 2t+1 of a
    partition hold same-label rows for t < PAIRS-1; singletons land in the
    final chunk pair. Returns (perm, pair_labels_a, pair_labels_b) where
    perm[p, k] = original row index placed at (p, k), and the label arrays
    are [P, PAIRS] giving the label of the row in chunks 2t / 2t+1."""
    order = np.argsort(lab, kind="stable")
    sorted_lab = lab[order]
    # split sorted runs into pairs + singles per class
    pairs = []      # (row_a, row_b) same label
    singles = []
    i = 0
    n = len(lab)
    while i < n:
        j = i
        while j < n and sorted_lab[j] == sorted_lab[i]:
            j += 1
        rows = order[i:j]
        m = (j - i) // 2 * 2
        for a in range(0, m, 2):
            pairs.append((rows[a], rows[a + 1]))
        if m < j - i:
            singles.append(rows[m])
        i = j
    # mixed pairs from singletons (count is even: sum of class counts even)
    for a in range(0, len(singles), 2):
        pairs.append((singles[a], singles[a + 1]))
    assert len(pairs) == P * PAIRS
    # clean pairs first, mixed last -> mixed all land in the final pair slot
    perm = np.empty((P, CHUNKS), dtype=np.int64)
    la = np.empty((P, PAIRS), dtype=np.int64)
    lb = np.empty((P, PAIRS), dtype=np.int64)
    idx = 0
    for t in range(PAIRS):
        for p in range(P):
            a, b = pairs[idx]
            idx += 1
            perm[p, 2 * t] = a
            perm[p, 2 * t + 1] = b
            la[p, t] = lab[a]
            lb[p, t] = lab[b]
    assert np.all(la[:, :PAIRS - 1] == lb[:, :PAIRS - 1])
    return perm, la, lb


def make_in_maps(feat, label):
    feat = np.asarray(feat, dtype=np.float32)
    lab = np.asarray(label).astype(np.int64)
    perm, la, lb = _pair_permutation(lab)

    cls = np.arange(NR, dtype=np.int64)[None, None, :]
    # shared matrices for pairs 0..PAIRS-2 come from la; the mixed pair
    # (t = PAIRS-1) contributes two matrices (la and lb).
    oh_arr = np.zeros((P, NMAT, NR), dtype=E4M3)
    oh_arr[:, :PAIRS, :] = (la[:, :, None] == cls).astype(E4M3)
    oh_arr[:, PAIRS, :] = (lb[:, PAIRS - 1, None] ==
                           cls[0]).astype(E4M3)
    oh_arr[:, :, NCLS] = E4M3(1.0)     # ones column -> s row

    hi = feat.astype(E4M3)
    lo = (feat - hi.astype(np.float32)).astype(E4M3)
    w = np.zeros((P, 1), dtype=np.float32)
    w[0:NCLS, 0] = -(1.0 + LAMDA)
    w[NCLS, 0] = LAMDA

    flat_perm = perm.reshape(-1)
    maps = []
    oh_flat = oh_arr.reshape(P * NMAT, NR)
    for m in range(N_CORES):
        csl = slice(m * DPC, (m + 1) * DPC)
        fhl = np.concatenate([hi[flat_perm][:, csl], lo[flat_perm][:, csl]],
                             axis=1)
        maps.append({"oh": oh_flat, "fhl": np.ascontiguousarray(fhl),
                     "wv": w})
    return maps


def kernel(feat, label, _trace=False):
    nc = _get_nc()
    in_maps = make_in_maps(feat, label)
    res = bass_utils.run_bass_kernel_spmd(
        nc, in_maps, core_ids=list(range(N_CORES)), trace=_trace)
    total = np.float64(0.0)
    for r in res.results:
        total += np.float64(r["out"]).sum()
    out = np.float32(total)
    if _trace:
        return out, res
    return out


# revision 13
# speedup vs baseline: 1.0349x; 1.0349x over previous
"""Trainium2 Bass kernel for nn_Contrast_loss (B=8192, D=256, 100 classes).

Math: with mask = -same + 0.5*(1-same) + I and same_ii = 1,
    loss = 0.5*||s||^2 - 1.5*sum_c ||g_c||^2 + sum_i ||f_i||^2
where s = sum_i f_i and g_c = sum_{i: label_i = c} f_i.

Every term decomposes over feature columns, so feat is sharded column-wise
across the 8 cores (32 columns each); the host sums the per-core partials.
No cross-core collective is needed.

Per core, everything runs through one fp8 DoubleRow matmul stream:
  - the host re-encodes label as one-hot fp8 (exact in fp8) with an extra
    all-ones column (computes s in the same matmul), and feat as an fp8
    hi/lo pair (hi = e4m3(f), lo = e4m3(f - hi); ~8-bit mantissa total).
  - the loss is invariant under row permutation, so the host pairs up
    same-label rows and places each pair in one (partition, chunk-pair)
    slot. Both k-subtiles of a DoubleRow matmul then share ONE one-hot
    matrix (stride-0 weight AP), halving the one-hot DMA. Odd class counts
    leave <=100 singletons, which exactly fill one final mixed pair that
    gets two distinct matrices (sum of per-class counts is even, so
    pairs + mixed slots == B/256 exactly).
  - the PE accumulates G = [onehot|1]^T @ [hi|lo] over 32 DoubleRow matmuls.
  - the diag term sum ||f_i||^2 = sum hi^2 + sum lo^2 (the 2*hi*lo cross
    term is ~2e-5 of the total, dropped) comes from Scalar
    square-accumulate passes over fhl, overlapped with the matmul stream.
  - tail: q_c = ||g_c||^2 on DVE reading PSUM, weighted and summed with the
    diag partials, GpSimd partition-reduce, one [1,1] DMA out; the host
    sums the 8 per-core scalars.
"""

import numpy as np
import ml_dtypes

import concourse.bacc as bacc
import concourse.bass as bass
import concourse.mybir as mybir
import concourse.tile as tile
from concourse import bass_utils

B = 8192
D = 256
N_CORES = 8
DPC = D // N_CORES          # 32 feature columns per core
P = 128                     # partitions
CHUNKS = B // P             # 64 row chunks of 128
PAIRS = CHUNKS // 2         # 32 chunk pairs
NMAT = PAIRS + 1            # 31 shared matrices + 2 for the mixed pair
N_GROUPS = 4
PPG = PAIRS // N_GROUPS     # 8 pairs per group
NCLS = 100                  # label values 0..99
NR = NCLS + 12              # one-hot cols + ones col + pad to mult-of-16 (dual-fp8 LDW)
LAMDA = 0.5

FP32 = mybir.dt.float32
BF16 = mybir.dt.bfloat16
FP8 = mybir.dt.float8e4
E4M3 = ml_dtypes.float8_e4m3

_CACHED_NC = None


def _build_nc():
    nc = bacc.Bacc("TRN2", target_bir_lowering=False, debug=False,
                   num_devices=N_CORES)

    oh_d = nc.dram_tensor("oh", [P * NMAT, NR], FP8, kind="ExternalInput")
    fhl_d = nc.dram_tensor("fhl", [B, 2 * DPC], FP8, kind="ExternalInput")
    w_d = nc.dram_tensor("wv", [P, 1], FP32, kind="ExternalInput")
    out_d = nc.dram_tensor("out", [1, 1], FP32, kind="ExternalOutput")

    with tile.TileContext(nc) as tc:
        with (
            tc.tile_pool(name="big", bufs=1) as big,
            tc.tile_pool(name="small", bufs=1) as small,
            tc.tile_pool(name="psum", bufs=1, space="PSUM") as psum,
        ):
            # Row r = p*CHUNKS + k lives at (partition p, chunk k).
            oh_t = big.tile([P, NMAT, NR], FP8)
            fhl_t = big.tile([P, CHUNKS, 2 * DPC], FP8)
            sq_t = big.tile([P, CHUNKS, 2 * DPC], BF16)
            dacc = small.tile([P, N_GROUPS], FP32)
            w_t = small.tile([P, 1], FP32)
            qq = small.tile([P, 1], FP32)

            psum_g = psum.tile([NR, 2 * DPC], FP32)

            nc.scalar.dma_start(w_t[:], w_d.rearrange("p c -> p c"))
            nc.vector.memset(qq[:], 0.0)

            oh_src = oh_d.rearrange("(p t) c -> p t c", p=P)
            fhl_src = fhl_d.rearrange("(p k) d -> p k d", p=P)

            def tslice(g):
                return slice(g * PPG, (g + 1) * PPG +
                             (1 if g == N_GROUPS - 1 else 0))

            def kslice(g):
                return slice(g * 2 * PPG, (g + 1) * 2 * PPG)

            # group 0 rides the gpsimd queue (served first) so the PE can
            # start early; the rest of oh goes on the sync queue in parallel.
            nc.gpsimd.dma_start(fhl_t[:, kslice(0), :], fhl_src[:, kslice(0), :])
            nc.gpsimd.dma_start(oh_t[:, tslice(0), :], oh_src[:, tslice(0), :])
            for g in range(1, N_GROUPS):
                nc.sync.dma_start(oh_t[:, tslice(g), :], oh_src[:, tslice(g), :])
                nc.gpsimd.dma_start(fhl_t[:, kslice(g), :],
                                    fhl_src[:, kslice(g), :])

            for g in range(N_GROUPS):
                ksl = kslice(g)
                # diag partials: Scalar square-accumulates most groups;
                # Vector covers one so the serial Scalar chain ends sooner.
                if g == 1:
                    nc.vector.tensor_mul(sq_t[:, ksl, :], fhl_t[:, ksl, :],
                                         fhl_t[:, ksl, :])
                    nc.vector.tensor_reduce(dacc[:, g:g + 1], sq_t[:, ksl, :],
                                            mybir.AxisListType.XY,
                                            mybir.AluOpType.add)
                else:
                    nc.scalar.activation(sq_t[:, ksl, :], fhl_t[:, ksl, :],
                                         mybir.ActivationFunctionType.Square,
                                         accum_out=dacc[:, g:g + 1])
                for t in range(g * PPG, (g + 1) * PPG):
                    if t < PAIRS - 1:
                        lhsT = oh_t[:, t, :].unsqueeze(1).broadcast_to(
                            [P, 2, NR])
                    else:
                        lhsT = oh_t[:, PAIRS - 1:PAIRS + 1, :]
                    nc.tensor.matmul(psum_g[:], lhsT,
                                     fhl_t[:, 2 * t:2 * t + 2, :],
                                     start=(t == 0), stop=(t == PAIRS - 1),
                                     perf_mode=mybir.MatmulPerfMode.DoubleRow)

            # G rows: 0..99 = [g_hi | g_lo] per class, 100 = [s_hi | s_lo]
            gh = small.tile([NR, DPC], FP32)
            nc.vector.tensor_copy(gh[:], psum_g[:, 0:DPC])
            gt = small.tile([NR, DPC], FP32)
            nc.vector.tensor_add(gt[:], gh[:], psum_g[:, DPC:2 * DPC])
            qsc = small.tile([NR, DPC], FP32)
            nc.vector.tensor_mul(qsc[:], gt[:], gt[:])
            nc.vector.tensor_reduce(qq[0:NR, 0:1], qsc[:],
                                    mybir.AxisListType.X, mybir.AluOpType.add)
            dsum = small.tile([P, 1], FP32)
            nc.vector.tensor_reduce(dsum[:], dacc[:], mybir.AxisListType.X,
                                    mybir.AluOpType.add)
            comb = small.tile([P, 1], FP32)
            nc.vector.tensor_mul(comb[:], qq[:], w_t[:])
            nc.vector.tensor_add(comb[:], comb[:], dsum[:])
            res_t = small.tile([1, 1], FP32)
            nc.gpsimd.tensor_reduce(res_t[:], comb[:], mybir.AxisListType.C,
                                    mybir.AluOpType.add)
            nc.sync.dma_start(out_d[:], res_t[:])

    nc.compile()
    return nc


def _get_nc():
    global _CACHED_NC
    if _CACHED_NC is None:
        _CACHED_NC = _build_nc()
    return _CACHED_NC


def _pair_permutation(lab):
    """Assign rows to (partition, chunk) slots so that chunks 2t, 2t+1 of a
    partition hold same-label rows for t < PAIRS-1; singletons land in the
    final chunk pair. Returns (perm, pair_labels_a, pair_labels_b) where
    perm[p, k] = original row index placed at (p, k), and the label arrays
    are [P, PAIRS] giving the label of the row in chunks 2t / 2t+1."""
    order = np.argsort(lab, kind="stable")
    sorted_lab = lab[order]
    # split sorted runs into pairs + singles per class
    pairs = []      # (row_a, row_b) same label
    singles = []
    i = 0
    n = len(lab)
    while i < n:
        j = i
        while j < n and sorted_lab[j] == sorted_lab[i]:
            j += 1
        rows = order[i:j]
        m = (j - i) // 2 * 2
        for a in range(0, m, 2):
            pairs.append((rows[a], rows[a + 1]))
        if m < j - i:
            singles.append(rows[m])
        i = j
    # mixed pairs from singletons (count is even: sum of class counts even)
    for a in range(0, len(singles), 2):
        pairs.append((singles[a], singles[a + 1]))
    assert len(pairs) == P * PAIRS
    # clean pairs first, mixed last -> mixed all land in the final pair slot
    perm = np.empty((P, CHUNKS), dtype=np.int64)
    la = np.empty((P, PAIRS), dtype=np.int64)
    lb = np.empty((P, PAIRS), dtype=np.int64)
    idx = 0
    for t in range(PAIRS):
        for p in range(P):
            a, b = pairs[idx]
            idx += 1
            perm[p, 2 * t] = a
            perm[p, 2 * t + 1] = b
            la[p, t] = lab[a]
            lb[p, t] = lab[b]
    assert np.all(la[:, :PAIRS - 1] == lb[:, :PAIRS - 1])
    return perm, la, lb


def make_in_maps(feat, label):
    feat = np.asarray(feat, dtype=np.float32)
    lab = np.asarray(label).astype(np.int64)
    perm, la, lb = _pair_permutation(lab)

    cls = np.arange(NR, dtype=np.int64)[None, None, :]
    # shared matrices for pairs 0..PAIRS-2 come from la; the mixed pair
    # (t = PAIRS-1) contributes two matrices (la and lb).
    oh_arr = np.zeros((P, NMAT, NR), dtype=E4M3)
    oh_arr[:, :PAIRS, :] = (la[:, :, None] == cls).astype(E4M3)
    oh_arr[:, PAIRS, :] = (lb[:, PAIRS - 1, None] ==
                           cls[0]).astype(E4M3)
    oh_arr[:, :, NCLS] = E4M3(1.0)     # ones column -> s row

    hi = feat.astype(E4M3)
    lo = (feat - hi.astype(np.float32)).astype(E4M3)
    w = np.zeros((P, 1), dtype=np.float32)
    w[0:NCLS, 0] = -(1.0 + LAMDA)
    w[NCLS, 0] = LAMDA

    flat_perm = perm.reshape(-1)
    maps = []
    oh_flat = oh_arr.reshape(P * NMAT, NR)
    for m in range(N_CORES):
        csl = slice(m * DPC, (m + 1) * DPC)
        fhl = np.concatenate([hi[flat_perm][:, csl], lo[flat_perm][:, csl]],
                             axis=1)
        maps.append({"oh": oh_flat, "fhl": np.ascontiguousarray(fhl),
                     "wv": w})
    return maps


def kernel(feat, label, _trace=False):
    nc = _get_nc()
    in_maps = make_in_maps(feat, label)
    res = bass_utils.run_bass_kernel_spmd(
        nc, in_maps, core_ids=list(range(N_CORES)), trace=_trace)
    total = np.float64(0.0)
    for r in res.results:
        total += np.float64(r["out"]).sum()
    out = np.float32(total)
    if _trace:
        return out, res
    return out


# revision 14
# speedup vs baseline: 1.1252x; 1.0873x over previous
"""Trainium2 Bass kernel for nn_Contrast_loss (B=8192, D=256, 100 classes).

Math: with mask = -same + 0.5*(1-same) + I and same_ii = 1,
    loss = 0.5*||s||^2 - 1.5*sum_c ||g_c||^2 + sum_i ||f_i||^2
where s = sum_i f_i and g_c = sum_{i: label_i = c} f_i.

Every term decomposes over feature columns, so feat is sharded column-wise
across the 8 cores (32 columns each); the host sums the per-core partials.
No cross-core collective is needed.

Per core, everything runs through one fp8 DoubleRow matmul stream:
  - the host re-encodes label as a one-hot fp8 matrix (exact in fp8) with an
    extra all-ones column (computes s in the same matmul), and feat as an
    fp8 hi/lo pair (hi = e4m3(f), lo = e4m3(f - hi); ~8-bit mantissa total).
  - the PE accumulates G = [onehot|1]^T @ [hi|lo] over 64 row chunks, two
    chunks per DoubleRow matmul.
  - the diag term sum ||f_i||^2 = sum hi^2 + sum lo^2 (the 2*hi*lo cross term
    is ~2e-5 of the total, dropped) comes from Scalar square-accumulate
    passes over fhl, overlapped with the matmul stream.
  - tail: q_c = ||g_c||^2 on DVE reading PSUM directly, weighted and summed
    with the diag partials into comb[128,1], which is DMA'd out; the host
    finishes the 128-lane + cross-core reduction.
"""

import numpy as np
import ml_dtypes

import concourse.bacc as bacc
import concourse.bass as bass
import concourse.mybir as mybir
import concourse.tile as tile
from concourse import bass_utils

B = 8192
D = 256
N_CORES = 8
DPC = D // N_CORES          # 32 feature columns per core
P = 128                     # partitions
CHUNKS = B // P             # 64 row chunks of 128
N_GROUPS = 4                # DMA / pipeline groups
CPG = CHUNKS // N_GROUPS    # 16 chunks per group
NCLS = 100                  # label values 0..99
NR = NCLS + 12              # one-hot cols + ones col + pad to mult-of-16 (dual-fp8 LDW)
LAMDA = 0.5

FP32 = mybir.dt.float32
BF16 = mybir.dt.bfloat16
FP8 = mybir.dt.float8e4
E4M3 = ml_dtypes.float8_e4m3

_CACHED_NC = None


def _build_nc():
    nc = bacc.Bacc("TRN2", target_bir_lowering=False, debug=False,
                   num_devices=N_CORES)

    oh_d = nc.dram_tensor("oh", [B, NR], FP8, kind="ExternalInput")
    fhl_d = nc.dram_tensor("fhl", [B, 2 * DPC], FP8, kind="ExternalInput")
    w_d = nc.dram_tensor("wv", [P, 1], FP32, kind="ExternalInput")
    out_d = nc.dram_tensor("out", [1, 1], FP32, kind="ExternalOutput")

    with tile.TileContext(nc) as tc:
        with (
            tc.tile_pool(name="big", bufs=1) as big,
            tc.tile_pool(name="small", bufs=1) as small,
            tc.tile_pool(name="psum", bufs=1, space="PSUM") as psum,
        ):
            # Row r = p*CHUNKS + k lives at (partition p, chunk k).
            oh_t = big.tile([P, CHUNKS, NR], FP8)
            fhl_t = big.tile([P, CHUNKS, 2 * DPC], FP8)
            sq_t = big.tile([P, CHUNKS, 2 * DPC], BF16)
            dacc = small.tile([P, N_GROUPS], FP32)
            w_t = small.tile([P, 1], FP32)
            qq = small.tile([P, 1], FP32)

            psum_g = psum.tile([NR, 2 * DPC], FP32)

            nc.scalar.dma_start(w_t[:], w_d.rearrange("p c -> p c"))
            nc.vector.memset(qq[:], 0.0)

            oh_src = oh_d.rearrange("(p k) c -> p k c", p=P)
            fhl_src = fhl_d.rearrange("(p k) d -> p k d", p=P)
            for g in range(N_GROUPS):
                ksl = slice(g * CPG, (g + 1) * CPG)
                nc.sync.dma_start(oh_t[:, ksl, :], oh_src[:, ksl, :])
                nc.gpsimd.dma_start(fhl_t[:, ksl, :], fhl_src[:, ksl, :])
                # diag partials on Scalar (overlapped with the PE stream)
                nc.scalar.activation(sq_t[:, ksl, :], fhl_t[:, ksl, :],
                                     mybir.ActivationFunctionType.Square,
                                     accum_out=dacc[:, g:g + 1])
                for k in range(g * CPG, (g + 1) * CPG, 2):
                    nc.tensor.matmul(psum_g[:], oh_t[:, k:k + 2, :],
                                     fhl_t[:, k:k + 2, :],
                                     start=(k == 0), stop=(k == CHUNKS - 2),
                                     perf_mode=mybir.MatmulPerfMode.DoubleRow)

            # G rows: 0..99 = [g_hi | g_lo] per class, 100 = [s_hi | s_lo]
            gh = small.tile([NR, DPC], FP32)
            nc.vector.tensor_copy(gh[:], psum_g[:, 0:DPC])
            gt = small.tile([NR, DPC], FP32)
            nc.vector.tensor_add(gt[:], gh[:], psum_g[:, DPC:2 * DPC])
            qsc = small.tile([NR, DPC], FP32)
            nc.vector.tensor_mul(qsc[:], gt[:], gt[:])
            nc.vector.tensor_reduce(qq[0:NR, 0:1], qsc[:],
                                    mybir.AxisListType.X, mybir.AluOpType.add)
            dsum = small.tile([P, 1], FP32)
            nc.vector.tensor_reduce(dsum[:], dacc[:], mybir.AxisListType.X,
                                    mybir.AluOpType.add)
            comb = small.tile([P, 1], FP32)
            nc.vector.tensor_mul(comb[:], qq[:], w_t[:])
            nc.vector.tensor_add(comb[:], comb[:], dsum[:])
            res_t = small.tile([1, 1], FP32)
            nc.gpsimd.tensor_reduce(res_t[:], comb[:], mybir.AxisListType.C,
                                    mybir.AluOpType.add)
            nc.sync.dma_start(out_d[:], res_t[:])

    nc.compile()
    return nc


def _get_nc():
    global _CACHED_NC
    if _CACHED_NC is None:
        _CACHED_NC = _build_nc()
    return _CACHED_NC


def make_in_maps(feat, label):
    feat = np.asarray(feat, dtype=np.float32)
    lab = np.asarray(label).astype(np.int32)
    oh = (lab[:, None] == np.arange(NR, dtype=np.int32)[None, :])
    oh = oh.astype(E4M3)
    oh[:, NCLS] = E4M3(1.0)            # ones column -> s row
    hi = feat.astype(E4M3)
    lo = (feat - hi.astype(np.float32)).astype(E4M3)
    w = np.zeros((P, 1), dtype=np.float32)
    w[0:NCLS, 0] = -(1.0 + LAMDA)
    w[NCLS, 0] = LAMDA
    maps = []
    for m in range(N_CORES):
        csl = slice(m * DPC, (m + 1) * DPC)
        fhl = np.concatenate([hi[:, csl], lo[:, csl]], axis=1)
        maps.append({"oh": oh, "fhl": np.ascontiguousarray(fhl), "wv": w})
    return maps


def kernel(feat, label, _trace=False):
    nc = _get_nc()
    in_maps = make_in_maps(feat, label)
    res = bass_utils.run_bass_kernel_spmd(
        nc, in_maps, core_ids=list(range(N_CORES)), trace=_trace)
    total = np.float64(0.0)
    for r in res.results:
        total += np.float64(r["out"]).sum()
    out = np.float32(total)
    if _trace:
        return out, res
    return out
